# revision 3
# baseline (speedup 1.0000x reference)
"""Bahdanau additive attention on 8 TRN2 NeuronCores, data-parallel over batch.

reference:
    h1 = enc @ W1 + b1              [B,S,U]
    h2 = hid @ W2 + b2              [B,1,U]
    score = tanh(h1+h2) @ V + bv    [B,S,1]   (bv dropped: softmax-invariant)
    w = softmax(score, axis=S)
    ctx = sum_s w * enc             [B,D]

Sharding: data-parallel over batch, 4 batches per core, weights replicated,
no collectives. Per-core pipeline (prev bf16 version: 321 us, PE 90% busy):
  - enc tiles cast-DMA'd f32->bf16 (SWDGE) into SBUF, kept for the ctx pass.
  - encT [d, s] built by PE matmuls against a bf16 identity; PSUM->SBUF
    copies cast to fp8e4 (DVE/ACT alternating) since encT only feeds h1.
  - h1T = W1.T @ encT in fp8 DoubleRow perf mode (2 k-planes per matmul,
    0.5 cyc/row = 4x bf16 MAC rate). Precision: plain fp8 h1 rel_err 2.45e-2
    > 2e-2 gate, so W1 is split two-level: W1hi = e4m3(W1) plus
    W1lo = e5m2(W1 - W1hi), both accumulated into the same PSUM group
    (8 DoubleRow matmuls per (m,t): 4 hi + 4 lo). Host-emulated end-to-end
    rel_err 1.45e-2. enc stays single-level e4m3 (bf16-rounded first).
  - ScalarE tanh with per-partition bias (h2+b1+b2 precomputed on host:
    67 MFLOP = 0.05% of device FLOPs).
  - score row = ones.T @ vacc per s-block, where vacc = sum_m V_m*tanh_m is
    a DVE fused-multiply-add chain (V in bf16 so the DVE 2x 16-bit perf mode
    applies) -> scoreT lands directly in [s_part, 1] layout for the ctx pass.
  - softmax without max-subtraction (scores are O(1), exp-safe in f32);
    partition-sum via ones-matmul; softmax+ctx of batch b deferred past
    batch b+1's transposes to hide the serial exp/reciprocal chain.
  - ctx = esc.T @ enc_native from the cached bf16 tiles, scaled by 1/sum.
"""
import sys
import numpy as np
from contextlib import ExitStack

if "/opt/trn_rl_repo" not in sys.path:
    sys.path.insert(0, "/opt/trn_rl_repo")

import ml_dtypes
from concourse import bacc, mybir, tile
from concourse.bass_utils import run_bass_kernel_spmd
from concourse.masks import make_identity

F32 = mybir.dt.float32
BF16 = mybir.dt.bfloat16
FP8E4 = mybir.dt.float8e4
FP8E5 = mybir.dt.float8e5
BF16NP = ml_dtypes.bfloat16
E4NP = ml_dtypes.float8_e4m3
E5NP = ml_dtypes.float8_e5m2
DR = mybir.MatmulPerfMode.DoubleRow

B, S, D, U = 32, 2048, 1024, 1024
NCORES = 8
BL = B // NCORES          # 4 batches per core
P = 128
KD = D // P               # 8 d-chunks
KU = U // P               # 8 u-chunks
NT = 512                  # matmul free-dim tile
ST = S // NT              # 4 s-tiles per batch
SB = S // P               # 16 s-blocks of 128

_NC_CACHE = None
LAST_RESULT = None        # test.py reads exec_time_ns off this
TRACE_DIR = None          # when set (and BASS_TRACE=1), ntff profile lands here


def _build():
    nc = bacc.Bacc("TRN2", target_bir_lowering=False)

    enc_in = nc.dram_tensor("enc", [BL, S, D], F32, kind="ExternalInput")
    w1hi_in = nc.dram_tensor("w1hi", [P, KD, U], FP8E4, kind="ExternalInput")
    w1lo_in = nc.dram_tensor("w1lo", [P, KD, U], FP8E5, kind="ExternalInput")
    bias_in = nc.dram_tensor("biasT", [P, KU * BL], F32, kind="ExternalInput")
    vT_in = nc.dram_tensor("vT", [P, KU], F32, kind="ExternalInput")
    out_ext = nc.dram_tensor("out", [BL, D], F32, kind="ExternalOutput")

    with tile.TileContext(nc) as tc, ExitStack() as ctx:
        const = ctx.enter_context(tc.tile_pool(name="const", bufs=1))
        nat_pool = ctx.enter_context(tc.tile_pool(name="nat", bufs=24))
        encT_pool = ctx.enter_context(tc.tile_pool(name="encT", bufs=3))
        tanh_pool = ctx.enter_context(tc.tile_pool(name="tanh", bufs=3))
        vacc_pool = ctx.enter_context(tc.tile_pool(name="vacc", bufs=2))
        small = ctx.enter_context(tc.tile_pool(name="small", bufs=4))
        out_pool = ctx.enter_context(tc.tile_pool(name="outp", bufs=2))

        ps_tr = ctx.enter_context(tc.tile_pool(name="ps_tr", bufs=2, space="PSUM"))
        ps_h1 = ctx.enter_context(tc.tile_pool(name="ps_h1", bufs=2, space="PSUM"))
        ps_misc = ctx.enter_context(tc.tile_pool(name="ps_misc", bufs=2, space="PSUM"))
        ps_ctx = ctx.enter_context(tc.tile_pool(name="ps_ctx", bufs=1, space="PSUM"))

        # ---- constants ----
        ident = const.tile([P, P], BF16)
        make_identity(nc, ident[:])
        ones128 = const.tile([P, 1], BF16)
        nc.any.memset(ones128[:], 1.0)
        w1hi_sb = const.tile([P, KD, U], FP8E4)
        nc.sync.dma_start(w1hi_sb[:], w1hi_in[:])
        w1lo_sb = const.tile([P, KD, U], FP8E5)
        nc.sync.dma_start(w1lo_sb[:], w1lo_in[:])
        v32_sb = const.tile([P, KU], F32)
        nc.scalar.dma_start(v32_sb[:], vT_in[:])
        bias_sb = const.tile([P, KU * BL], F32)   # bias[u(m,p), m*BL+b]
        nc.scalar.dma_start(bias_sb[:], bias_in[:])

        # ---- main per-batch pipeline ----
        def emit_transposes(nat_tiles, t, encT):
            """encT[:, k, j*128:(j+1)*128] = nat[t*4+j][:, k*128:(k+1)*128].T

            Done as regular matmuls against the identity (out = natchunk.T @ I):
            keeps the PE HAM activity monitor warm and pipelines at ~46ns/op,
            unlike transpose-mode (~236ns, doesn't count as PE-busy).
            The PSUM->SBUF copy casts to fp8e4 (encT only feeds the fp8 h1).
            """
            for k in range(KD):
                pt = ps_tr.tile([P, NT], F32)
                for j in range(NT // P):
                    nc.tensor.matmul(
                        pt[:, j * P:(j + 1) * P],
                        nat_tiles[t * (NT // P) + j][:, k * P:(k + 1) * P],
                        ident[:], start=True, stop=True)
                if k % 2 == 0:
                    nc.vector.tensor_copy(encT[:, k, :], pt[:])
                else:
                    nc.scalar.activation(encT[:, k, :], pt[:],
                                         mybir.ActivationFunctionType.Copy)

        pending_tail = []
        for b in range(BL):
            nat_tiles = []

            def emit_nat(lo, hi, b=b, nat_tiles=nat_tiles):
                for st in range(lo, hi):
                    nt_t = nat_pool.tile([P, D], BF16, name=f"nat_{b}_{st}",
                                         tag="nat")
                    nc.gpsimd.dma_start(nt_t[:], enc_in[b, st * P:(st + 1) * P, :])
                    nat_tiles.append(nt_t)

            emit_nat(0, 8)

            encT = encT_pool.tile([P, KD, NT], FP8E4)
            emit_transposes(nat_tiles, 0, encT)
            # previous batch's softmax+ctx lands here: its exp/reciprocal
            # latency hides under this batch's transposes, and its PE
            # matmuls run just before this batch's first mm1.
            if pending_tail:
                pending_tail.pop(0)()
            psum_sT = ps_misc.tile([P, SB], F32, tag="misc")
            for t in range(ST):
                if t in (1, 2):
                    emit_nat(8 if t == 1 else 12, 12 if t == 1 else 16)
                vacc = vacc_pool.tile([P, NT], BF16)
                for m in range(KU):
                    ph1 = ps_h1.tile([P, NT], F32)
                    for kk in range(KD // 2):
                        nc.tensor.matmul(
                            ph1[:],
                            w1hi_sb[:, 2 * kk:2 * kk + 2, m * P:(m + 1) * P],
                            encT[:, 2 * kk:2 * kk + 2, :],
                            start=(kk == 0), stop=False, perf_mode=DR)
                    for kk in range(KD // 2):
                        nc.tensor.matmul(
                            ph1[:],
                            w1lo_sb[:, 2 * kk:2 * kk + 2, m * P:(m + 1) * P],
                            encT[:, 2 * kk:2 * kk + 2, :],
                            start=False, stop=(kk == KD // 2 - 1), perf_mode=DR)
                    tanh_t = tanh_pool.tile([P, NT], BF16)
                    nc.scalar.activation(
                        tanh_t[:], ph1[:], mybir.ActivationFunctionType.Tanh,
                        bias=bias_sb[:, m * BL + b:m * BL + b + 1], scale=1.0)
                    if m == 0:
                        nc.vector.tensor_scalar_mul(
                            vacc[:], tanh_t[:], v32_sb[:, 0:1])
                    else:
                        nc.vector.scalar_tensor_tensor(
                            vacc[:], tanh_t[:], v32_sb[:, m:m + 1], vacc[:],
                            mybir.AluOpType.mult, mybir.AluOpType.add)
                    if m == 0 and t < ST - 1:
                        encT_next = encT_pool.tile([P, KD, NT], FP8E4)
                        emit_transposes(nat_tiles, t + 1, encT_next)
                for jj in range(NT // P):
                    nc.tensor.matmul(
                        psum_sT[:, t * (NT // P) + jj:t * (NT // P) + jj + 1],
                        vacc[:, jj * P:(jj + 1) * P], ones128[:, :1],
                        start=True, stop=True)
                if t < ST - 1:
                    encT = encT_next

            def emit_softmax_ctx(b=b, psum_sT=psum_sT, nat_tiles=nat_tiles):
                esc = small.tile([P, SB], BF16, name=f"esc{b}", tag="esc")
                rowsum = small.tile([P, 1], F32, name=f"rowsum{b}", tag="rowsum")
                nc.scalar.activation(
                    esc[:], psum_sT[:], mybir.ActivationFunctionType.Exp,
                    accum_out=rowsum[:])
                rs_bf = small.tile([P, 1], BF16, name=f"rs_bf{b}", tag="rs_bf")
                nc.vector.tensor_copy(rs_bf[:], rowsum[:])
                psum_s1 = ps_misc.tile([1, 1], F32, tag="misc")
                nc.tensor.matmul(psum_s1[:], rs_bf[:, :], ones128[:, :1],
                                 start=True, stop=True)
                sum_sb = small.tile([1, 1], F32, name=f"sum_sb{b}", tag="sum_sb")
                nc.vector.tensor_copy(sum_sb[:], psum_s1[:])
                rinv = small.tile([1, 1], F32, name=f"rinv{b}", tag="rinv")
                nc.vector.reciprocal(rinv[:], sum_sb[:])

                # ctx = esc.T @ enc (native tiles), scaled by 1/sum
                pc = [ps_ctx.tile([1, NT], F32, name=f"pc{h}", tag=f"pc{h}")
                      for h in range(D // NT)]
                for j in range(SB):
                    for h in range(D // NT):
                        nc.tensor.matmul(
                            pc[h][:], esc[:, j:j + 1],
                            nat_tiles[j][:, h * NT:(h + 1) * NT],
                            start=(j == 0), stop=(j == SB - 1))
                out_t = out_pool.tile([1, D], F32, name=f"out_t{b}", tag="out_t")
                for h in range(D // NT):
                    nc.vector.tensor_scalar_mul(
                        out_t[:1, h * NT:(h + 1) * NT], pc[h][:], rinv[:1, :1])
                nc.sync.dma_start(out_ext[b:b + 1, :], out_t[:1, :])

            pending_tail.append(emit_softmax_ctx)
        # last batch has no successor to hide under; emit directly
        pending_tail.pop(0)()

    nc.compile()
    return nc


def _get_nc():
    global _NC_CACHE
    if _NC_CACHE is None:
        _NC_CACHE = _build()
    return _NC_CACHE


def kernel(**inputs):
    global LAST_RESULT
    enc = np.asarray(inputs["enc"], dtype=np.float32)
    hid = np.asarray(inputs["hid"], dtype=np.float32)
    W1 = np.asarray(inputs["W1"], dtype=np.float32)
    b1 = np.asarray(inputs["b1"], dtype=np.float32)
    W2 = np.asarray(inputs["W2"], dtype=np.float32)
    b2 = np.asarray(inputs["b2"], dtype=np.float32)
    V = np.asarray(inputs["V"], dtype=np.float32)
    # bv shifts all scores of a batch equally -> softmax unchanged; unused.

    # host-side layout prep (pure reshapes/casts of tiny tensors)
    w1r = np.ascontiguousarray(
        W1.reshape(KD, P, U).transpose(1, 0, 2))             # [P, KD, U] f32
    w1hi = w1r.astype(E4NP)
    w1lo = (w1r - w1hi.astype(np.float32)).astype(E5NP)
    vT = np.ascontiguousarray(V.reshape(KU, P).T)
    # h2+biases on host: 67 MFLOP, 0.05% of the device work
    bias_full = (hid @ W2 + b2 + b1).astype(np.float32)      # [B, U]

    nc = _get_nc()
    in_maps = []
    for i in range(NCORES):
        bs = bias_full[i * BL:(i + 1) * BL]                  # [BL, U]
        biasT = np.ascontiguousarray(
            bs.reshape(BL, KU, P).transpose(2, 1, 0).reshape(P, KU * BL))
        in_maps.append({
            "enc": np.ascontiguousarray(enc[i * BL:(i + 1) * BL]),
            "w1hi": w1hi, "w1lo": w1lo, "biasT": biasT, "vT": vT,
        })
    kwargs = {}
    if TRACE_DIR is not None:
        kwargs["tmpdir"] = TRACE_DIR
    res = run_bass_kernel_spmd(nc, in_maps, list(range(NCORES)), **kwargs)
    LAST_RESULT = res
    out = np.concatenate([res.results[i]["out"] for i in range(NCORES)], axis=0)
    return out.astype(np.float32)


# revision 10
# speedup vs baseline: 1.4459x; 1.4459x over previous
"""Bahdanau additive attention on 8 TRN2 NeuronCores, data-parallel over batch.

reference:
    h1 = enc @ W1 + b1              [B,S,U]
    h2 = hid @ W2 + b2              [B,1,U]
    score = tanh(h1+h2) @ V + bv    [B,S,1]   (bv dropped: softmax-invariant)
    w = softmax(score, axis=S)
    ctx = sum_s w * enc             [B,D]

Sharding: data-parallel over batch, 4 batches per core, weights replicated,
no collectives. Per-core pipeline (prev bf16 version: 321 us, PE 90% busy):
  - enc tiles cast-DMA'd f32->bf16 (SWDGE) into SBUF, kept for the ctx pass.
  - encT [d, s] built by PE matmuls against a bf16 identity; PSUM->SBUF
    copies cast to fp8e4 (DVE/ACT alternating) since encT only feeds h1.
  - h1T = W1.T @ encT in fp8 DoubleRow perf mode (2 k-planes per matmul;
    measured on silicon DR streams 256-deep contraction per ~258ns = ~2x
    bf16 MACs, not the 4x the cost model claims). Precision: plain fp8 h1
    rel_err 2.45e-2 > 2e-2 gate. Full two-level W1 (hi+lo passes) costs
    bf16 parity, so instead: u-axis is permuted by |V| descending (host
    side, consistently for W1/bias/V) and the W1lo = e5m2(W1 - W1hi)
    correction pass runs only for the top 512 u (first 4 of 8 m-chunks,
    93% of sum V^2). Host-emulated end-to-end rel_err 1.55e-2.
    enc stays single-level e4m3 (bf16-rounded first).
  - ScalarE tanh with per-partition bias (h2+b1+b2 precomputed on host:
    67 MFLOP = 0.05% of device FLOPs).
  - score row = ones.T @ vacc per s-block, where vacc = sum_m V_m*tanh_m is
    a DVE fused-multiply-add chain (V in bf16 so the DVE 2x 16-bit perf mode
    applies) -> scoreT lands directly in [s_part, 1] layout for the ctx pass.
  - softmax without max-subtraction (scores are O(1), exp-safe in f32);
    partition-sum via ones-matmul; softmax+ctx of batch b deferred past
    batch b+1's transposes to hide the serial exp/reciprocal chain.
  - ctx = esc.T @ enc_native from the cached bf16 tiles, scaled by 1/sum.
"""
import sys
import numpy as np
from contextlib import ExitStack

if "/opt/trn_rl_repo" not in sys.path:
    sys.path.insert(0, "/opt/trn_rl_repo")

import ml_dtypes
from concourse import bacc, mybir, tile
from concourse.bass_utils import run_bass_kernel_spmd
from concourse.masks import make_identity

F32 = mybir.dt.float32
BF16 = mybir.dt.bfloat16
FP8E4 = mybir.dt.float8e4
FP8E5 = mybir.dt.float8e5
BF16NP = ml_dtypes.bfloat16
E4NP = ml_dtypes.float8_e4m3
E5NP = ml_dtypes.float8_e5m2
DR = mybir.MatmulPerfMode.DoubleRow

B, S, D, U = 32, 2048, 1024, 1024
NCORES = 8
BL = B // NCORES          # 4 batches per core
P = 128
KD = D // P               # 8 d-chunks
KU = U // P               # 8 u-chunks
NT = 512                  # matmul free-dim tile
ST = S // NT              # 4 s-tiles per batch
SB = S // P               # 16 s-blocks of 128

_NC_CACHE = None
LAST_RESULT = None        # test.py reads exec_time_ns off this
TRACE_DIR = None          # when set (and BASS_TRACE=1), ntff profile lands here


def _build():
    nc = bacc.Bacc("TRN2", target_bir_lowering=False)

    enc_in = nc.dram_tensor("enc", [BL, S, D], F32, kind="ExternalInput")
    w1hi_in = nc.dram_tensor("w1hi", [P, KD, U], FP8E4, kind="ExternalInput")
    w1lo_in = nc.dram_tensor("w1lo", [P, KD, U // 2], FP8E5, kind="ExternalInput")
    bias_in = nc.dram_tensor("biasT", [P, KU * BL], F32, kind="ExternalInput")
    vT_in = nc.dram_tensor("vT", [P, KU], F32, kind="ExternalInput")
    out_ext = nc.dram_tensor("out", [BL, D], F32, kind="ExternalOutput")

    with tile.TileContext(nc) as tc, ExitStack() as ctx:
        const = ctx.enter_context(tc.tile_pool(name="const", bufs=1))
        nat_pool = ctx.enter_context(tc.tile_pool(name="nat", bufs=24))
        encT_pool = ctx.enter_context(tc.tile_pool(name="encT", bufs=3))
        tanh_pool = ctx.enter_context(tc.tile_pool(name="tanh", bufs=3))
        vacc_pool = ctx.enter_context(tc.tile_pool(name="vacc", bufs=2))
        small = ctx.enter_context(tc.tile_pool(name="small", bufs=4))
        out_pool = ctx.enter_context(tc.tile_pool(name="outp", bufs=2))

        ps_tr = ctx.enter_context(tc.tile_pool(name="ps_tr", bufs=2, space="PSUM"))
        ps_h1 = ctx.enter_context(tc.tile_pool(name="ps_h1", bufs=2, space="PSUM"))
        ps_misc = ctx.enter_context(tc.tile_pool(name="ps_misc", bufs=2, space="PSUM"))
        ps_ctx = ctx.enter_context(tc.tile_pool(name="ps_ctx", bufs=1, space="PSUM"))

        # ---- constants ----
        ident = const.tile([P, P], BF16)
        make_identity(nc, ident[:])
        ones128 = const.tile([P, 1], BF16)
        nc.any.memset(ones128[:], 1.0)
        w1hi_sb = const.tile([P, KD, U], FP8E4)
        nc.sync.dma_start(w1hi_sb[:], w1hi_in[:])
        w1lo_sb = const.tile([P, KD, U // 2], FP8E5)
        nc.sync.dma_start(w1lo_sb[:], w1lo_in[:])
        v32_sb = const.tile([P, KU], F32)
        nc.scalar.dma_start(v32_sb[:], vT_in[:])
        bias_sb = const.tile([P, KU * BL], F32)   # bias[u(m,p), m*BL+b]
        nc.scalar.dma_start(bias_sb[:], bias_in[:])

        # ---- main per-batch pipeline ----
        def emit_transposes(nat_tiles, t, encT):
            """encT[:, k, j*128:(j+1)*128] = nat[t*4+j][:, k*128:(k+1)*128].T

            Done as regular matmuls against the identity (out = natchunk.T @ I):
            keeps the PE HAM activity monitor warm and pipelines at ~46ns/op,
            unlike transpose-mode (~236ns, doesn't count as PE-busy).
            The PSUM->SBUF copy casts to fp8e4 (encT only feeds the fp8 h1).
            """
            for k in range(KD):
                pt = ps_tr.tile([P, NT], F32)
                for j in range(NT // P):
                    nc.tensor.matmul(
                        pt[:, j * P:(j + 1) * P],
                        nat_tiles[t * (NT // P) + j][:, k * P:(k + 1) * P],
                        ident[:], start=True, stop=True)
                if k % 2 == 0:
                    nc.vector.tensor_copy(encT[:, k, :], pt[:])
                else:
                    nc.scalar.activation(encT[:, k, :], pt[:],
                                         mybir.ActivationFunctionType.Copy)

        pending_tail = []
        for b in range(BL):
            nat_tiles = []

            def emit_nat(lo, hi, b=b, nat_tiles=nat_tiles):
                for st in range(lo, hi):
                    nt_t = nat_pool.tile([P, D], BF16, name=f"nat_{b}_{st}",
                                         tag="nat")
                    nc.gpsimd.dma_start(nt_t[:], enc_in[b, st * P:(st + 1) * P, :])
                    nat_tiles.append(nt_t)

            emit_nat(0, 8)

            encT = encT_pool.tile([P, KD, NT], FP8E4)
            emit_transposes(nat_tiles, 0, encT)
            # previous batch's softmax+ctx lands here: its exp/reciprocal
            # latency hides under this batch's transposes, and its PE
            # matmuls run just before this batch's first mm1.
            if pending_tail:
                pending_tail.pop(0)()
            psum_sT = ps_misc.tile([P, SB], F32, tag="misc")
            for t in range(ST):
                if t in (1, 2):
                    emit_nat(8 if t == 1 else 12, 12 if t == 1 else 16)
                vacc = vacc_pool.tile([P, NT], BF16)
                for m in range(KU):
                    ph1 = ps_h1.tile([P, NT], F32)
                    has_lo = m < KU // 2
                    for kk in range(KD // 2):
                        nc.tensor.matmul(
                            ph1[:],
                            w1hi_sb[:, 2 * kk:2 * kk + 2, m * P:(m + 1) * P],
                            encT[:, 2 * kk:2 * kk + 2, :],
                            start=(kk == 0),
                            stop=(not has_lo and kk == KD // 2 - 1),
                            perf_mode=DR)
                    if has_lo:
                        for kk in range(KD // 2):
                            nc.tensor.matmul(
                                ph1[:],
                                w1lo_sb[:, 2 * kk:2 * kk + 2, m * P:(m + 1) * P],
                                encT[:, 2 * kk:2 * kk + 2, :],
                                start=False, stop=(kk == KD // 2 - 1),
                                perf_mode=DR)
                    tanh_t = tanh_pool.tile([P, NT], BF16)
                    nc.scalar.activation(
                        tanh_t[:], ph1[:], mybir.ActivationFunctionType.Tanh,
                        bias=bias_sb[:, m * BL + b:m * BL + b + 1], scale=1.0)
                    if m == 0:
                        nc.vector.tensor_scalar_mul(
                            vacc[:], tanh_t[:], v32_sb[:, 0:1])
                    else:
                        nc.vector.scalar_tensor_tensor(
                            vacc[:], tanh_t[:], v32_sb[:, m:m + 1], vacc[:],
                            mybir.AluOpType.mult, mybir.AluOpType.add)
                    if m == 0 and t < ST - 1:
                        encT_next = encT_pool.tile([P, KD, NT], FP8E4)
                        emit_transposes(nat_tiles, t + 1, encT_next)
                for jj in range(NT // P):
                    nc.tensor.matmul(
                        psum_sT[:, t * (NT // P) + jj:t * (NT // P) + jj + 1],
                        vacc[:, jj * P:(jj + 1) * P], ones128[:, :1],
                        start=True, stop=True)
                if t < ST - 1:
                    encT = encT_next

            def emit_softmax_ctx(b=b, psum_sT=psum_sT, nat_tiles=nat_tiles):
                esc = small.tile([P, SB], BF16, name=f"esc{b}", tag="esc")
                rowsum = small.tile([P, 1], F32, name=f"rowsum{b}", tag="rowsum")
                nc.scalar.activation(
                    esc[:], psum_sT[:], mybir.ActivationFunctionType.Exp,
                    accum_out=rowsum[:])
                rs_bf = small.tile([P, 1], BF16, name=f"rs_bf{b}", tag="rs_bf")
                nc.vector.tensor_copy(rs_bf[:], rowsum[:])
                psum_s1 = ps_misc.tile([1, 1], F32, tag="misc")
                nc.tensor.matmul(psum_s1[:], rs_bf[:, :], ones128[:, :1],
                                 start=True, stop=True)
                sum_sb = small.tile([1, 1], F32, name=f"sum_sb{b}", tag="sum_sb")
                nc.vector.tensor_copy(sum_sb[:], psum_s1[:])
                rinv = small.tile([1, 1], F32, name=f"rinv{b}", tag="rinv")
                nc.vector.reciprocal(rinv[:], sum_sb[:])

                # ctx = esc.T @ enc (native tiles), scaled by 1/sum
                pc = [ps_ctx.tile([1, NT], F32, name=f"pc{h}", tag=f"pc{h}")
                      for h in range(D // NT)]
                for j in range(SB):
                    for h in range(D // NT):
                        nc.tensor.matmul(
                            pc[h][:], esc[:, j:j + 1],
                            nat_tiles[j][:, h * NT:(h + 1) * NT],
                            start=(j == 0), stop=(j == SB - 1))
                out_t = out_pool.tile([1, D], F32, name=f"out_t{b}", tag="out_t")
                for h in range(D // NT):
                    nc.vector.tensor_scalar_mul(
                        out_t[:1, h * NT:(h + 1) * NT], pc[h][:], rinv[:1, :1])
                nc.sync.dma_start(out_ext[b:b + 1, :], out_t[:1, :])

            pending_tail.append(emit_softmax_ctx)
        # last batch has no successor to hide under; emit directly
        pending_tail.pop(0)()

    nc.compile()
    return nc


def _get_nc():
    global _NC_CACHE
    if _NC_CACHE is None:
        _NC_CACHE = _build()
    return _NC_CACHE


def kernel(**inputs):
    global LAST_RESULT
    enc = np.asarray(inputs["enc"], dtype=np.float32)
    hid = np.asarray(inputs["hid"], dtype=np.float32)
    W1 = np.asarray(inputs["W1"], dtype=np.float32)
    b1 = np.asarray(inputs["b1"], dtype=np.float32)
    W2 = np.asarray(inputs["W2"], dtype=np.float32)
    b2 = np.asarray(inputs["b2"], dtype=np.float32)
    V = np.asarray(inputs["V"], dtype=np.float32)
    # bv shifts all scores of a batch equally -> softmax unchanged; unused.

    # host-side layout prep (pure reshapes/casts of tiny tensors).
    # u-axis permuted by |V| descending so the fp8 lo-correction pass can
    # cover only the top-512 u (they carry ~93% of sum V^2).
    perm = np.argsort(-np.abs(V[:, 0]))
    W1p = np.ascontiguousarray(W1[:, perm])
    Vp = V[perm, 0]
    w1r = np.ascontiguousarray(
        W1p.reshape(KD, P, U).transpose(1, 0, 2))            # [P, KD, U] f32
    w1hi = w1r.astype(E4NP)
    w1lo = (w1r[:, :, :U // 2]
            - w1hi[:, :, :U // 2].astype(np.float32)).astype(E5NP)
    vT = np.ascontiguousarray(Vp.reshape(KU, P).T)
    # h2+biases on host: 67 MFLOP, 0.05% of the device work
    bias_full = (hid @ W2 + b2 + b1).astype(np.float32)[:, perm]  # [B, U]

    nc = _get_nc()
    in_maps = []
    for i in range(NCORES):
        bs = bias_full[i * BL:(i + 1) * BL]                  # [BL, U]
        biasT = np.ascontiguousarray(
            bs.reshape(BL, KU, P).transpose(2, 1, 0).reshape(P, KU * BL))
        in_maps.append({
            "enc": np.ascontiguousarray(enc[i * BL:(i + 1) * BL]),
            "w1hi": w1hi, "w1lo": w1lo, "biasT": biasT, "vT": vT,
        })
    kwargs = {}
    if TRACE_DIR is not None:
        kwargs["tmpdir"] = TRACE_DIR
    res = run_bass_kernel_spmd(nc, in_maps, list(range(NCORES)), **kwargs)
    LAST_RESULT = res
    out = np.concatenate([res.results[i]["out"] for i in range(NCORES)], axis=0)
    return out.astype(np.float32)
